# revision 6
# baseline (speedup 1.0000x reference)
"""MoE (8 experts, top-2, D=H=1024, N=1024 tokens) on 8 TRN2 NeuronCores.

Strategy: host-side routing (router GEMM is 1024x8 — trivial), expert-parallel
on device: core e runs expert e's SwiGLU on its routed tokens (padded to CAP)
plus a 128-token slice of the shared expert. Activations are kept transposed
([D, T] layout) so every matmul uses weights as the stationary operand with no
on-device transposes. Matmuls run in bf16 with fp32 PSUM accumulation; the
host casts weights to bf16 once and combines per-expert outputs with the
routing scores.
"""
import numpy as np
import ml_dtypes

from concourse import bacc, bass, tile, mybir
from concourse.bass_utils import run_bass_kernel_spmd

P = 128
D = 1024
H = 1024
E = 8
K = 2
N = 1024
CAP = 288  # max routed tokens per expert is 278 for this problem's fixed seed
# (deterministic inputs; any overflow is computed exactly on the host spill path)
KD = D // P
KH = H // P
F32 = mybir.dt.float32
BF16 = mybir.dt.bfloat16
BF = ml_dtypes.bfloat16

_COMPILED = None


def _build():
    nc = bacc.Bacc(None, target_bir_lowering=False)

    w1_d = nc.dram_tensor("w1", (D, H), BF16, kind="ExternalInput")
    w3_d = nc.dram_tensor("w3", (D, H), BF16, kind="ExternalInput")
    w2_d = nc.dram_tensor("w2", (H, D), BF16, kind="ExternalInput")
    sw1_d = nc.dram_tensor("sw1", (D, H), BF16, kind="ExternalInput")
    sw3_d = nc.dram_tensor("sw3", (D, H), BF16, kind="ExternalInput")
    sw2_d = nc.dram_tensor("sw2", (H, D), BF16, kind="ExternalInput")
    xt_d = nc.dram_tensor("xt", (D, CAP), BF16, kind="ExternalInput")
    xs_d = nc.dram_tensor("xs", (D, P), BF16, kind="ExternalInput")
    ye_d = nc.dram_tensor("ye", (D, CAP), F32, kind="ExternalOutput")
    ys_d = nc.dram_tensor("ys", (D, P), F32, kind="ExternalOutput")

    with tile.TileContext(nc) as tc:
        with (
            tc.tile_pool(name="w", bufs=1) as wpool,
            tc.tile_pool(name="x", bufs=1) as xpool,
            tc.tile_pool(name="h", bufs=1) as hpool,
            tc.tile_pool(name="stage", bufs=3) as spool,
            tc.tile_pool(name="out", bufs=3) as opool,
            tc.tile_pool(name="pp1", bufs=2, space="PSUM") as pp1,
            tc.tile_pool(name="pp3", bufs=2, space="PSUM") as pp3,
            tc.tile_pool(name="ppy", bufs=2, space="PSUM") as ppy,
            tc.tile_pool(name="const", bufs=1) as cpool,
        ):
            bias0 = cpool.tile([P, 1], F32)
            nc.any.memset(bias0[:], 0.0)

            def swiglu(T, xT, a1, a3, a2, yT, pfx):
                w1t, w3t, w2t = [], [], []
                for kd in range(KD):
                    t1 = wpool.tile([P, H], BF16, tag=f"{pfx}w1_{kd}")
                    nc.sync.dma_start(t1[:], a1[kd * P : (kd + 1) * P, :])
                    w1t.append(t1)
                    t3 = wpool.tile([P, H], BF16, tag=f"{pfx}w3_{kd}")
                    nc.sync.dma_start(t3[:], a3[kd * P : (kd + 1) * P, :])
                    w3t.append(t3)
                for kh in range(KH):
                    t2 = wpool.tile([P, D], BF16, tag=f"{pfx}w2_{kh}")
                    nc.sync.dma_start(t2[:], a2[kh * P : (kh + 1) * P, :])
                    w2t.append(t2)

                xts = []
                for kd in range(KD):
                    xb = xpool.tile([P, T], BF16, tag=f"{pfx}x_{kd}")
                    nc.sync.dma_start(xb[:], xT[kd * P : (kd + 1) * P, :])
                    xts.append(xb)

                hts = []
                for mh in range(KH):
                    p1 = pp1.tile([P, T], F32, tag="p1")
                    p3 = pp3.tile([P, T], F32, tag="p3")
                    for kd in range(KD):
                        nc.tensor.matmul(
                            p1[:],
                            w1t[kd][:, mh * P : (mh + 1) * P],
                            xts[kd][:],
                            start=(kd == 0),
                            stop=(kd == KD - 1),
                        )
                    for kd in range(KD):
                        nc.tensor.matmul(
                            p3[:],
                            w3t[kd][:, mh * P : (mh + 1) * P],
                            xts[kd][:],
                            start=(kd == 0),
                            stop=(kd == KD - 1),
                        )
                    sl = spool.tile([P, T], F32, tag="silu")
                    nc.scalar.activation(
                        sl[:], p1[:], mybir.ActivationFunctionType.Silu, bias=bias0[:]
                    )
                    hb = hpool.tile([P, T], BF16, tag=f"{pfx}h_{mh}")
                    nc.vector.tensor_mul(hb[:], sl[:], p3[:])
                    hts.append(hb)

                for md in range(KD):
                    py = ppy.tile([P, T], F32, tag="py")
                    for kh in range(KH):
                        nc.tensor.matmul(
                            py[:],
                            w2t[kh][:, md * P : (md + 1) * P],
                            hts[kh][:],
                            start=(kh == 0),
                            stop=(kh == KH - 1),
                        )
                    ot = opool.tile([P, T], F32, tag="ot")
                    nc.vector.tensor_copy(ot[:], py[:])
                    nc.sync.dma_start(yT[md * P : (md + 1) * P, :], ot[:])

            swiglu(CAP, xt_d, w1_d, w3_d, w2_d, ye_d, "e")
            swiglu(P, xs_d, sw1_d, sw3_d, sw2_d, ys_d, "s")

    nc.compile()
    return nc


def _route(x_flat, router_w, expert_bias):
    logits = x_flat @ router_w.astype(np.float32)
    logits = logits - logits.max(-1, keepdims=True)
    sc = np.exp(logits)
    sc /= sc.sum(-1, keepdims=True)
    sel = np.argsort(-(sc + expert_bias[None, :].astype(np.float32)),
                     axis=-1, kind="stable")[:, :K]
    tsc = np.take_along_axis(sc, sel, axis=-1)
    return sel, tsc


def kernel(x, router_w, expert_bias, w1, w2, w3, sw1, sw2, sw3):
    global _COMPILED
    x = np.asarray(x, np.float32)
    x_flat = np.ascontiguousarray(x.reshape(N, D))
    sel, tsc = _route(x_flat, np.asarray(router_w), np.asarray(expert_bias))

    if _COMPILED is None:
        _COMPILED = _build()
    nc = _COMPILED

    in_maps = []
    ids_l, wts_l, cnt_l = [], [], []
    sw1b = np.asarray(sw1).astype(BF)
    sw3b = np.asarray(sw3).astype(BF)
    sw2b = np.asarray(sw2).astype(BF)
    spill = []  # (expert, ids, wts) computed on host if CAP ever overflows
    for e in range(E):
        mask = sel == e  # [N, K]
        rows = mask.any(-1)
        ids = np.nonzero(rows)[0]
        wts = tsc[mask]  # aligned with ids (row-major, <=1 hit per row)
        cnt = ids.shape[0]
        if cnt > CAP:
            spill.append((e, ids[CAP:], wts[CAP:]))
            ids, wts, cnt = ids[:CAP], wts[:CAP], CAP
        ids_p = np.zeros(CAP, np.int64)
        ids_p[:cnt] = ids
        xtT = np.ascontiguousarray(x_flat[ids_p].T.astype(BF))
        xsT = np.ascontiguousarray(x_flat[e * P : (e + 1) * P].T.astype(BF))
        in_maps.append(
            {
                "w1": np.asarray(w1[e]).astype(BF),
                "w3": np.asarray(w3[e]).astype(BF),
                "w2": np.asarray(w2[e]).astype(BF),
                "sw1": sw1b,
                "sw3": sw3b,
                "sw2": sw2b,
                "xt": xtT,
                "xs": xsT,
            }
        )
        ids_l.append(ids)
        wts_l.append(wts)
        cnt_l.append(cnt)

    res = run_bass_kernel_spmd(nc, in_maps, core_ids=list(range(E))).results

    out = np.zeros((N, D), np.float32)
    for e in range(E):
        cnt = cnt_l[e]
        yeT = np.asarray(res[e]["ye"], np.float32)  # [D, CAP]
        out[ids_l[e]] += wts_l[e][:, None].astype(np.float32) * yeT.T[:cnt]
        ysT = np.asarray(res[e]["ys"], np.float32)  # [D, P]
        out[e * P : (e + 1) * P] += ysT.T
    for e, ids, wts in spill:  # rare overflow path: exact swiglu on host
        xe = x_flat[ids]
        h = xe @ np.asarray(w1[e], np.float32)
        h = (h / (1.0 + np.exp(-h))) * (xe @ np.asarray(w3[e], np.float32))
        out[ids] += wts[:, None].astype(np.float32) * (
            h @ np.asarray(w2[e], np.float32)
        )
    return out.reshape(1, N, D)
